# revision 52
# baseline (speedup 1.0000x reference)
"""ArcMarginLoss distributed Trainium2 kernel (8 NeuronCores, class-sharded).

Math (equivalent to the reference, no arccos needed):
  x_hat = x / max(||x||, eps);  w_hat = w / max(||w||, eps)
  cos[i,c] = x_hat[i] . w_hat[c]
  For the label class only: m_i = cos(arccos(clip(c_i)) + M)
                                = clip(c_i)*cos(M) - sin(M)*sqrt(1-clip(c_i)^2)
  logits = S*cos except S*m_i at the label
  nll_i = logsumexp_c(logits[i]) - S*m_i
        = ln( sum_c exp(S*cos[i,c]) - exp(S*c_i) + exp(S*m_i) ) - S*m_i
  out = mean_i nll_i
S*cos is in [-16, 16] so no max-subtraction is needed for a stable sum-exp.

Distribution: classes padded 32000 -> 32768 with zero rows and sharded
contiguously, 4096 per core.  Each core computes its local sum-exp plus its
owned rows' correction terms; four [128,32] f32 AllReduces (issued as each
quarter of the rows completes) combine
  A[i] = sum_c exp(S*cos) - exp(S*c_i) + exp(S*m_i)   (pads add exp(0)=1 each)
  B[i] = S*m_i
then every core computes mean(ln(A - 768) - B).

Implementation notes:
- The bulk cosine matmul runs in fp8e4 (e4m3) with MatmulPerfMode.DoubleRow:
  each instruction contracts K=256 (two 128-blocks).  w_hat is pre-scaled by
  16 so its entries (~N(0, 1/512)) sit in e4m3's normal range; x is cast raw
  (entries ~N(0,1)).  The per-row exp scale 1/||x|| absorbs S/16 exactly
  (S=16).  The label-correction path stays fp32, so rel err stays ~1e-4.
- x is staged twice from the host: natural [N,D] (for row norms and the
  label-dot path) and transposed [D,N] (so the fp8 stationary tiles are a
  single strided load + one DVE cast — no on-device transpose for x).
- w is loaded naturally, normalized, scaled to bf16, transposed in pairs on
  the DMA transpose engine, and cast to fp8 on DVE.
- exp is fused on the Scalar engine: activation(Exp, scale=1/||x||_row as a
  per-partition AP, accum_out=row sums) over [128,2048] psum tiles (4 banks,
  double-buffered = all 8), written back in place to psum.  The Scalar
  engine runs ONLY Exp until the final Ln, so exactly one ACT table set is
  live for the whole steady state (Ln/Exp sit in different sets; mixing
  them costs a 1.3us table reload each way).
- all rsqrt/sqrt run on DVE as Newton iterations from a constant initial
  guess (the row/class norms concentrate tightly around their means, and
  the iteration is self-correcting: 3 iterations from a 30%-error guess
  reach ~1e-6), so no Ln/Rsqrt tables are needed.
- label-row gathers trail the main loop by one x group instead of leading
  it, so their SWDGE descriptors and dot products land in the matmul-bound
  phase instead of the startup phase.
- x rows/w classes are packed 4/8 per partition ((p a) layout) for 8-16KB
  DMA descriptors; the class permutation is harmless (softmax sums classes)
  and the row permutation is undone on the host via lab/msk layout.
"""

import math
import sys

sys.path.insert(0, "/opt/trn_rl_repo")

import ml_dtypes
import numpy as np

from concourse import bacc, bass, mybir, tile
from concourse.bass_utils import run_bass_kernel_spmd

f32 = mybir.dt.float32
f8 = mybir.dt.float8e4
bf16 = mybir.dt.bfloat16
i32 = mybir.dt.int32

N, D, C = 8192, 512, 32000
NCORES = 8
CPAD = 32768            # padded class count (8 * 4096)
CS = CPAD // NCORES     # classes per core
P = 128                 # partitions
RT = N // P             # row tiles (64)
CT = CS // 512          # class tiles of 512 (8)
NPAD = float(CPAD - C)  # zero-pad classes, each contributes exp(0)=1

S_SCALE = 16.0
M_MARGIN = 0.2
EPS = 1e-7
COS_M = math.cos(M_MARGIN)
SIN_M = math.sin(M_MARGIN)
SS_FLOOR = 1e-24        # max(ss, floor) emulates torch F.normalize eps=1e-12
W8 = 16.0               # w_hat fp8 pre-scale; folded into the exp row scale

_CACHE = {}


def _build(ncores=NCORES):
    nc = bacc.Bacc("TRN2", target_bir_lowering=False, debug=False,
                   num_devices=ncores)
    x_d = nc.dram_tensor("x", [N, D], bf16, kind="ExternalInput")
    xT_d = nc.dram_tensor("xT", [D, N], bf16, kind="ExternalInput")
    w_d = nc.dram_tensor("w", [CS, D], bf16, kind="ExternalInput")
    wT_d = nc.dram_tensor("wT", [D, CS], bf16, kind="ExternalInput")
    lab_d = nc.dram_tensor("lab", [P, RT], i32, kind="ExternalInput")
    msk_d = nc.dram_tensor("msk", [P, RT], f32, kind="ExternalInput")
    out_d = nc.dram_tensor("out", [1, 1], f32, kind="ExternalOutput")

    mult = mybir.AluOpType.mult
    add = mybir.AluOpType.add
    sub = mybir.AluOpType.subtract
    amax = mybir.AluOpType.max
    amin = mybir.AluOpType.min
    Exp = mybir.ActivationFunctionType.Exp
    Ln = mybir.ActivationFunctionType.Ln
    DR = mybir.MatmulPerfMode.DoubleRow
    NW = CS // P       # 32 weight chunks of 128 classes
    XG = RT // 4       # 16 x groups (512 rows each)
    WG = 4             # w groups (1024 classes each, 8 chunks)
    QB = RT // 4       # quarter of the row-tile columns (16)

    with tile.TileContext(nc) as tc:
        with tc.tile_pool(name="persist", bufs=1) as persist, \
             tc.tile_pool(name="dram", bufs=1, space="DRAM") as dram, \
             tc.tile_pool(name="io", bufs=3) as io, \
             tc.tile_pool(name="big", bufs=3) as big, \
             tc.tile_pool(name="wgp", bufs=8) as wgp, \
             tc.tile_pool(name="xhp", bufs=8) as xhp, \
             tc.tile_pool(name="small", bufs=4) as small, \
             tc.tile_pool(name="pmm", bufs=2, space="PSUM") as pmm:

            def T(shape, name, dtype=f32):
                return persist.tile(shape, dtype, name=name)

            ones = T([P, 1], "ones")
            nc.vector.memset(ones[:], 1.0)

            labs = T([P, RT], "labs", dtype=i32)
            nc.gpsimd.dma_start(out=labs[:], in_=lab_d[:, :])
            msks = T([P, RT], "msks")
            nc.gpsimd.dma_start(out=msks[:], in_=msk_d[:, :])

            sumexp = T([P, RT], "sumexp")
            cdot = T([P, RT], "cdot")
            ctl = T([P, RT], "ctl")
            marg = T([P, RT], "marg")
            aloc = T([P, RT], "aloc")
            bloc = T([P, RT], "bloc")
            ssw_all = T([P, NW], "ssw_all")
            ssx_all = T([P, RT], "ssx_all")
            ssg_all = T([P, RT], "ssg_all")
            dotg_all = T([P, RT], "dotg_all")
            sxs_all = T([P, RT], "sxs_all")   # 1 / ||x_row||
            wsc_all = T([P, NW], "wsc_all")   # 16 / ||w_class||
            ar_ins = [dram.tile([P, 2 * QB], f32, name=f"ar_in{q}")
                      for q in range(4)]
            ar_outs = [dram.tile([P, 2 * QB], f32, name=f"ar_out{q}")
                       for q in range(4)]


            def rsqrt_newton(dst, src, width, final_scale=None, iters=2):
                # rsqrt on DVE: fast-inverse-sqrt bit seed (scale-free,
                # ~3.4% error) + Newton; 2 iterations reach ~5e-6.  Keeps
                # Ln/Rsqrt tables off the Scalar engine entirely.
                y = small.tile([P, width], f32, name="nwy")
                yi = y[:].bitcast(i32)
                nc.vector.tensor_scalar(
                    out=yi, in0=src.bitcast(i32), scalar1=1, scalar2=None,
                    op0=mybir.AluOpType.logical_shift_right)
                nc.vector.tensor_scalar(
                    out=yi, in0=yi, scalar1=-1, scalar2=0x5F3759DF,
                    op0=mult, op1=add)
                t = small.tile([P, width], f32, name="nwt")
                for it in range(iters):
                    nc.vector.tensor_tensor(out=t[:], in0=y[:], in1=y[:],
                                            op=mult)
                    nc.vector.tensor_tensor(out=t[:], in0=t[:], in1=src,
                                            op=mult)
                    nc.vector.tensor_scalar(out=t[:], in0=t[:],
                                            scalar1=-0.5, scalar2=1.5,
                                            op0=mult, op1=add)
                    if it == iters - 1:
                        if final_scale is not None:
                            nc.vector.scalar_tensor_tensor(
                                out=dst, in0=y[:], scalar=final_scale,
                                in1=t[:], op0=mult, op1=mult)
                        else:
                            nc.vector.tensor_tensor(out=dst, in0=y[:],
                                                    in1=t[:], op=mult)
                    else:
                        nc.vector.tensor_tensor(out=y[:], in0=y[:],
                                                in1=t[:], op=mult)

            # fp8 transposed, normalized, 16x-scaled w (class order permuted;
            # softmax is order-invariant), one tile per 512-class ct block:
            # whT8[ct][p, dc, c] = 16*w_hat[class(ct,c), dc*128 + p].
            whT8 = [T([P, 4, 512], f"whT8_{ct}", dtype=f8)
                    for ct in range(CT)]

            xh8_tiles = {}
            xt_tiles = {}
            xtT_tiles = {}
            xnat_tiles = {}
            blk_starts = {1: 0, 3: 2, 7: 4, 11: 8, 15: 12}

            def emit_xT_load(g2):
                # transposed stationary source: two x groups per DMA.
                # xh8[g][p, dc, m] = x[row(m) = g*512 + m, dc*128 + p] fp8
                xtT2 = io.tile([P, 4, 1024], bf16, name="xtT2", tag="xtT2",
                               bufs=2)
                xtT_tiles[g2] = xtT2
                nc.sync.dma_start(
                    out=xtT2[:],
                    in_=xT_d.rearrange("(dc p) r -> p dc r", p=P)[
                        :, :, g2 * 1024:(g2 + 1) * 1024])

            def emit_xcast(g2):
                xtT2 = xtT_tiles.pop(g2)
                for b in range(2):
                    g = 2 * g2 + b
                    xh8 = xhp.tile([P, 4, 512], f8, name="xh8")
                    xh8_tiles[g] = xh8
                    nc.vector.tensor_copy(
                        out=xh8[:], in_=xtT2[:, :, b * 512:(b + 1) * 512])

            def emit_xnat_load(g2):
                # natural layout for row norms + label dots
                xt2 = io.tile([P, 8, D], bf16, name="xt2", tag="xt2",
                              bufs=3)
                xnat_tiles[g2] = xt2
                nc.sync.dma_start(
                    out=xt2[:].rearrange("p (b a) d -> p b a d", b=2),
                    in_=x_d[g2 * 8 * P:(g2 + 1) * 8 * P, :].rearrange(
                        "(b p a) d -> p b a d", b=2, a=4))

            def emit_xnorm(g2):
                xt2 = xnat_tiles.pop(g2)
                for b in range(2):
                    g = 2 * g2 + b
                    xt_tiles[g] = (xt2, 4 * b)
                    for a in range(4):
                        t = g * 4 + a
                        xjk = big.tile([P, D], bf16, name="junk", bufs=2)
                        nc.vector.scalar_tensor_tensor(
                            out=xjk[:], in0=xt2[:, 4 * b + a], scalar=1.0,
                            in1=xt2[:, 4 * b + a],
                            op0=mult, op1=mult,
                            accum_out=ssx_all[:, t:t + 1])

            def emit_gather(g):
                xt2, boff = xt_tiles.pop(g)
                for a in range(4):
                    t = g * 4 + a
                    wg_t = wgp.tile([P, D], bf16, name="wg")
                    nc.gpsimd.indirect_dma_start(
                        out=wg_t[:], out_offset=None, in_=w_d[:, :],
                        in_offset=bass.IndirectOffsetOnAxis(
                            ap=labs[:, t:t + 1], axis=0))
                    gjk = big.tile([P, D], bf16, name="gjunk", bufs=2)
                    nc.vector.scalar_tensor_tensor(
                        out=gjk[:], in0=wg_t[:], scalar=1.0, in1=wg_t[:],
                        op0=mult, op1=mult,
                        accum_out=ssg_all[:, t:t + 1])
                    gjk2 = big.tile([P, D], bf16, name="gjunk2", bufs=2)
                    nc.vector.scalar_tensor_tensor(
                        out=gjk2[:], in0=wg_t[:], scalar=1.0,
                        in1=xt2[:, boff + a],
                        op0=mult, op1=mult,
                        accum_out=dotg_all[:, t:t + 1])

            def emit_sxs_batch(g):
                g_lo = blk_starts[g]
                blk = slice(g_lo * 4, (g + 1) * 4)
                nblk = (g + 1 - g_lo) * 4
                rsqrt_newton(sxs_all[:, blk], ssx_all[:, blk], nblk)

            def emit_quarter(qt):
                # corrections + allreduce for rows of column quarter qt
                cl = slice(qt * QB, (qt + 1) * QB)
                gsc = small.tile([P, QB], f32, name="gsc")
                rsqrt_newton(gsc[:], ssg_all[:, cl], QB)
                cd = cdot[:, cl]
                nc.vector.tensor_tensor(out=cd, in0=dotg_all[:, cl],
                                        in1=gsc[:], op=mult)
                nc.vector.tensor_tensor(out=cd, in0=cd, in1=sxs_all[:, cl],
                                        op=mult)
                nc.vector.tensor_scalar(out=ctl[:, cl], in0=cd,
                                        scalar1=(-1.0 + EPS),
                                        scalar2=(1.0 - EPS),
                                        op0=amax, op1=amin)
                negc2 = small.tile([P, QB], f32, name="negc2")
                nc.vector.scalar_tensor_tensor(out=negc2[:], in0=ctl[:, cl],
                                               scalar=-1.0, in1=ctl[:, cl],
                                               op0=mult, op1=mult)
                uu = small.tile([P, QB], f32, name="uu")
                nc.vector.tensor_scalar_add(out=uu[:], in0=negc2[:],
                                            scalar1=1.0)
                # sinsq = sin(M) * sqrt(uu) = sin(M) * uu * rsqrt(uu)
                ru = small.tile([P, QB], f32, name="ru")
                rsqrt_newton(ru[:], uu[:], QB)
                sinsq = small.tile([P, QB], f32, name="sinsq")
                nc.vector.scalar_tensor_tensor(out=sinsq[:], in0=uu[:],
                                               scalar=SIN_M, in1=ru[:],
                                               op0=mult, op1=mult)
                nc.vector.scalar_tensor_tensor(out=marg[:, cl],
                                               in0=ctl[:, cl],
                                               scalar=COS_M, in1=sinsq[:],
                                               op0=mult, op1=sub)
                e1 = small.tile([P, QB], f32, name="e1")
                nc.scalar.activation(out=e1[:], in_=marg[:, cl], func=Exp,
                                     scale=S_SCALE)
                e2 = small.tile([P, QB], f32, name="e2")
                nc.scalar.activation(out=e2[:], in_=ctl[:, cl], func=Exp,
                                     scale=S_SCALE)
                d12 = small.tile([P, QB], f32, name="d12")
                nc.vector.scalar_tensor_tensor(out=d12[:], in0=e1[:],
                                               scalar=1.0, in1=e2[:],
                                               op0=mult, op1=sub)
                corr = small.tile([P, QB], f32, name="corr")
                nc.vector.tensor_tensor(out=corr[:], in0=d12[:],
                                        in1=msks[:, cl], op=mult)
                nc.vector.tensor_tensor(out=aloc[:, cl], in0=sumexp[:, cl],
                                        in1=corr[:], op=add)
                nc.vector.scalar_tensor_tensor(out=bloc[:, cl],
                                               in0=marg[:, cl],
                                               scalar=S_SCALE,
                                               in1=msks[:, cl],
                                               op0=mult, op1=mult)
                ar_i, ar_o = ar_ins[qt], ar_outs[qt]
                nc.gpsimd.dma_start(out=ar_i[:, 0:QB], in_=aloc[:, cl])
                nc.gpsimd.dma_start(out=ar_i[:, QB:2 * QB],
                                    in_=bloc[:, cl])
                nc.gpsimd.collective_compute(
                    "AllReduce", add,
                    replica_groups=[list(range(ncores))],
                    ins=[ar_i[:].opt()], outs=[ar_o[:].opt()])

            # hoist the first 4 x groups (transposed halves first — they
            # gate the first matmuls); natural-layout norms are emitted
            # after the W chain so the DVE queue front stays on the
            # first-matmul critical path
            emit_xT_load(0)
            emit_xT_load(1)
            emit_xnat_load(0)
            emit_xnat_load(1)
            emit_xcast(0)
            emit_xcast(1)

            taccs_all = {}

            def emit_block_pass(tiles, ct4):
                for t in tiles:
                    if t not in taccs_all:
                        taccs_all[t] = small.tile([P, 2], f32,
                                                  name="accs", bufs=36)
                for t in tiles:
                    g2, a = t // 4, t % 4
                    ps = pmm.tile([P, 2048], f32, name="ps")
                    for q in range(4):
                        ct = ct4 * 4 + q
                        for b in range(2):
                            nc.tensor.matmul(
                                out=ps[:, q * 512:(q + 1) * 512],
                                lhsT=xh8_tiles[g2][:, 2 * b:2 * b + 2,
                                                   a * 128:(a + 1) * 128],
                                rhs=whT8[ct][:, 2 * b:2 * b + 2, :],
                                start=(b == 0), stop=(b == 1),
                                perf_mode=DR)
                    nc.scalar.activation(
                        out=ps[:], in_=ps[:], func=Exp,
                        scale=sxs_all[:, t:t + 1],
                        accum_out=taccs_all[t][:, ct4:ct4 + 1])

            def emit_block_reduce(tiles):
                for t in tiles:
                    acc = taccs_all.pop(t)
                    nc.vector.tensor_tensor(
                        out=sumexp[:, t:t + 1], in0=acc[:, 0:1],
                        in1=acc[:, 1:2], op=add)

            # ---- stage W: load natural w for class norms and transposed w
            #      for the matmul operand; scale columns to fp8 via a
            #      partition-broadcast multiply (no on-device transposes) ----
            wscrow = T([1, CS], "wscrow")
            for gw in range(WG):
                wt = io.tile([P, 8, D], bf16, name="wt", tag="wt", bufs=2)
                # class c = gw*1024 + p*8 + a  -> 8KB contiguous/partition
                nc.scalar.dma_start(
                    out=wt[:],
                    in_=w_d[gw * 8 * P:(gw + 1) * 8 * P, :].rearrange(
                        "(p a) d -> p a d", a=8))
                wTt = io.tile([P, 4, 1024], bf16, name="wTt", tag="wTt",
                              bufs=2)
                nc.sync.dma_start(
                    out=wTt[:],
                    in_=wT_d.rearrange("(dc p) c -> p dc c", p=P)[
                        :, :, gw * 1024:(gw + 1) * 1024])
                for a in range(8):
                    j = gw * 8 + a
                    wjk = big.tile([P, D], bf16, name="junk", bufs=2)
                    nc.vector.scalar_tensor_tensor(
                        out=wjk[:], in0=wt[:, a], scalar=1.0, in1=wt[:, a],
                        op0=mult, op1=mult,
                        accum_out=ssw_all[:, j:j + 1])
                gsl = slice(gw * 8, (gw + 1) * 8)
                wssc = small.tile([P, 8], f32, name="wssc")
                nc.vector.tensor_scalar_max(out=wssc[:],
                                            in0=ssw_all[:, gsl],
                                            scalar1=SS_FLOOR)
                rsqrt_newton(wsc_all[:, gsl], wssc[:], 8, final_scale=W8)
                # scatter this group's scales into the class-ordered row,
                # then physically replicate it across partitions
                nc.gpsimd.dma_start(
                    out=wscrow[0:1, gw * 1024:(gw + 1) * 1024].rearrange(
                        "one (p a) -> one p a", p=P),
                    in_=wsc_all[:, gsl])
                wscb = big.tile([P, 1024], f32, name="wscb", bufs=2)
                nc.gpsimd.partition_broadcast(
                    wscb[:], wscrow[0:1, gw * 1024:(gw + 1) * 1024])
                for h in range(2):
                    ct = gw * 2 + h
                    for dc in range(4):
                        nc.vector.tensor_tensor(
                            out=whT8[ct][:, dc, :],
                            in0=wTt[:, dc, h * 512:(h + 1) * 512],
                            in1=wscb[:, h * 512:(h + 1) * 512],
                            op=mult)
                if gw == 1:
                    emit_xnorm(0)
                    emit_sxs_batch(1)
                    emit_block_pass(list(range(8)), 0)
            emit_xnorm(1)
            emit_sxs_batch(3)
            emit_block_pass(list(range(8)), 1)
            emit_block_reduce(list(range(8)))



            # ---- main loop: x loads 4 groups ahead, gathers 1 behind,
            #      sxs batches 2 iterations ahead of their block ----
            batch_at = {5: 7, 9: 11, 13: 15}
            blk2 = {3: 2, 7: 4, 11: 8, 15: 12}
            qtr_at = {5: 0, 9: 1, 13: 2}
            for g in range(XG):
                if g % 2 == 0 and g + 4 < XG:
                    g2n = (g + 4) // 2
                    emit_xT_load(g2n)
                    emit_xnat_load(g2n)
                    emit_xcast(g2n)
                    emit_xnorm(g2n)
                if g in batch_at:
                    emit_sxs_batch(batch_at[g])
                if g in blk2:
                    tiles = list(range(blk2[g] * 4, (g + 1) * 4))
                    for ct4 in range(2):
                        emit_block_pass(tiles, ct4)
                    emit_block_reduce(tiles)
                if g >= 1:
                    emit_gather(g - 1)
                if g in qtr_at:
                    emit_quarter(qtr_at[g])
            emit_gather(XG - 1)
            emit_quarter(3)

            # ---- combine quarters and reduce to the scalar mean.  The
            # q0-q2 math is emitted first so it runs (after the exp stream,
            # one Ln table load) while quarter 3's AllReduce is in flight;
            # only q3's [P,16] math trails the final AR. ----
            gg = T([P, 2 * RT], "gg")
            nllq = T([P, RT], "nllq")
            rsums = T([P, 4], "rsums")
            for qt in range(4):
                ql = slice(qt * QB, (qt + 1) * QB)
                nc.gpsimd.dma_start(out=gg[:, qt * QB:(qt + 1) * QB],
                                    in_=ar_outs[qt][:, 0:QB])
                nc.gpsimd.dma_start(
                    out=gg[:, RT + qt * QB:RT + (qt + 1) * QB],
                    in_=ar_outs[qt][:, QB:2 * QB])
                atq = small.tile([P, QB], f32, name="atq")
                nc.vector.tensor_scalar_add(out=atq[:], in0=gg[:, ql],
                                            scalar1=-NPAD)
                lnaq = small.tile([P, QB], f32, name="lnaq")
                nc.scalar.activation(out=lnaq[:], in_=atq[:], func=Ln)
                nc.vector.scalar_tensor_tensor(
                    out=nllq[:, ql], in0=lnaq[:], scalar=1.0,
                    in1=gg[:, RT + qt * QB:RT + (qt + 1) * QB],
                    op0=mult, op1=sub)
                nc.vector.reduce_sum(out=rsums[:, qt:qt + 1],
                                     in_=nllq[:, ql],
                                     axis=mybir.AxisListType.X)
            rsum = T([P, 1], "rsum")
            nc.vector.reduce_sum(out=rsum[:], in_=rsums[:],
                                 axis=mybir.AxisListType.X)
            pf = pmm.tile([P, 2048], f32, name="ps")
            nc.tensor.matmul(out=pf[:1, :1], lhsT=rsum[:, :1],
                             rhs=ones[:, :1], start=True, stop=True)
            res = T([1, 1], "res")
            nc.vector.tensor_scalar_mul(out=res[:], in0=pf[:1, :1],
                                        scalar1=1.0 / float(N))
            nc.gpsimd.dma_start(out=out_d[:, :], in_=res[:])

    nc.compile()
    return nc


def _get_nc():
    if "nc" not in _CACHE:
        _CACHE["nc"] = _build()
    return _CACHE["nc"]


def kernel(prev_output, weight, labels, **trace_kwargs):
    bf = ml_dtypes.bfloat16
    x = np.ascontiguousarray(prev_output, dtype=np.float32)
    xb16 = x.astype(bf)
    xT16 = x.T.astype(bf)
    w = np.ascontiguousarray(weight, dtype=np.float32)
    lab = np.asarray(labels).astype(np.int64)

    wpad = np.zeros((CPAD, D), dtype=bf)
    wpad[:C] = w.astype(bf)

    in_maps = []
    for k in range(NCORES):
        lo = k * CS
        loc = (lab - lo).astype(np.int64)
        own = (loc >= 0) & (loc < CS)
        locc = np.clip(loc, 0, CS - 1).astype(np.int32)
        # row r = g*512 + p*4 + a maps to [p, t=g*4+a]
        lab2 = locc.reshape(RT // 4, P, 4).transpose(1, 0, 2).reshape(P, RT)
        msk2 = own.astype(np.float32).reshape(RT // 4, P, 4) \
                  .transpose(1, 0, 2).reshape(P, RT)
        in_maps.append({
            "x": xb16,
            "xT": xT16,
            "w": wpad[lo:lo + CS],
            "wT": np.ascontiguousarray(wpad[lo:lo + CS].T),
            "lab": np.ascontiguousarray(lab2),
            "msk": np.ascontiguousarray(msk2),
        })

    nc = _get_nc()
    res = run_bass_kernel_spmd(nc, in_maps, core_ids=list(range(NCORES)),
                               **trace_kwargs)
    if trace_kwargs:
        _CACHE["last_results"] = res
    return np.float32(res.results[0]["out"].reshape(())[()])


if __name__ == "__main__":
    rng = np.random.default_rng(0)
    x = rng.standard_normal((N, D), dtype=np.float32)
    w = rng.standard_normal((C, D), dtype=np.float32) * 0.01
    lab = rng.integers(0, C, N)
    got = kernel(x, w, lab)
    xh = x / np.maximum(np.linalg.norm(x, axis=1, keepdims=True), 1e-12)
    wh = w / np.maximum(np.linalg.norm(w, axis=1, keepdims=True), 1e-12)
    cos = (xh @ wh.T).astype(np.float64)
    th = np.arccos(np.clip(cos[np.arange(N), lab], -1 + EPS, 1 - EPS))
    ml = np.cos(th + M_MARGIN)
    logits = cos * S_SCALE
    tgt = ml * S_SCALE
    lse = np.log(np.exp(logits).sum(1) - np.exp(logits[np.arange(N), lab])
                 + np.exp(tgt))
    want = (lse - tgt).mean()
    print("got", got, "want", want, "relerr", abs(got - want) / abs(want))


# revision 53
# speedup vs baseline: 1.0535x; 1.0535x over previous
"""ArcMarginLoss distributed Trainium2 kernel (8 NeuronCores, class-sharded).

Math (equivalent to the reference, no arccos needed):
  x_hat = x / max(||x||, eps);  w_hat = w / max(||w||, eps)
  cos[i,c] = x_hat[i] . w_hat[c]
  For the label class only: m_i = cos(arccos(clip(c_i)) + M)
                                = clip(c_i)*cos(M) - sin(M)*sqrt(1-clip(c_i)^2)
  logits = S*cos except S*m_i at the label
  nll_i = logsumexp_c(logits[i]) - S*m_i
        = ln( sum_c exp(S*cos[i,c]) - exp(S*c_i) + exp(S*m_i) ) - S*m_i
  out = mean_i nll_i
S*cos is in [-16, 16] so no max-subtraction is needed for a stable sum-exp.

Distribution: classes padded 32000 -> 32768 with zero rows and sharded
contiguously, 4096 per core.  Each core computes its local sum-exp plus its
owned rows' correction terms; four [128,32] f32 AllReduces (issued as each
quarter of the rows completes) combine
  A[i] = sum_c exp(S*cos) - exp(S*c_i) + exp(S*m_i)   (pads add exp(0)=1 each)
  B[i] = S*m_i
then every core computes mean(ln(A - 768) - B).

Implementation notes:
- The bulk cosine matmul runs in fp8e4 (e4m3) with MatmulPerfMode.DoubleRow:
  each instruction contracts K=256 (two 128-blocks).  w_hat is pre-scaled by
  16 so its entries (~N(0, 1/512)) sit in e4m3's normal range; x is cast raw
  (entries ~N(0,1)).  The per-row exp scale 1/||x|| absorbs S/16 exactly
  (S=16).  The label-correction path stays fp32, so rel err stays ~1e-4.
- x is staged twice from the host: natural [N,D] (for row norms and the
  label-dot path) and transposed [D,N] (so the fp8 stationary tiles are a
  single strided load + one DVE cast — no on-device transpose for x).
- w is loaded naturally, normalized, scaled to bf16, transposed in pairs on
  the DMA transpose engine, and cast to fp8 on DVE.
- exp is fused on the Scalar engine: activation(Exp, scale=1/||x||_row as a
  per-partition AP, accum_out=row sums) over [128,2048] psum tiles (4 banks,
  double-buffered = all 8), written back in place to psum.  The Scalar
  engine runs ONLY Exp until the final Ln, so exactly one ACT table set is
  live for the whole steady state (Ln/Exp sit in different sets; mixing
  them costs a 1.3us table reload each way).
- all rsqrt/sqrt run on DVE as Newton iterations from a constant initial
  guess (the row/class norms concentrate tightly around their means, and
  the iteration is self-correcting: 3 iterations from a 30%-error guess
  reach ~1e-6), so no Ln/Rsqrt tables are needed.
- label-row gathers trail the main loop by one x group instead of leading
  it, so their SWDGE descriptors and dot products land in the matmul-bound
  phase instead of the startup phase.
- x rows/w classes are packed 4/8 per partition ((p a) layout) for 8-16KB
  DMA descriptors; the class permutation is harmless (softmax sums classes)
  and the row permutation is undone on the host via lab/msk layout.
"""

import math
import sys

sys.path.insert(0, "/opt/trn_rl_repo")

import ml_dtypes
import numpy as np

from concourse import bacc, bass, mybir, tile
from concourse.bass_utils import run_bass_kernel_spmd

f32 = mybir.dt.float32
f8 = mybir.dt.float8e4
bf16 = mybir.dt.bfloat16
i32 = mybir.dt.int32

N, D, C = 8192, 512, 32000
NCORES = 8
CPAD = 32768            # padded class count (8 * 4096)
CS = CPAD // NCORES     # classes per core
P = 128                 # partitions
RT = N // P             # row tiles (64)
CT = CS // 512          # class tiles of 512 (8)
NPAD = float(CPAD - C)  # zero-pad classes, each contributes exp(0)=1

S_SCALE = 16.0
M_MARGIN = 0.2
EPS = 1e-7
COS_M = math.cos(M_MARGIN)
SIN_M = math.sin(M_MARGIN)
SS_FLOOR = 1e-24        # max(ss, floor) emulates torch F.normalize eps=1e-12
W8 = 16.0               # w_hat fp8 pre-scale; folded into the exp row scale

_CACHE = {}


def _build(ncores=NCORES):
    nc = bacc.Bacc("TRN2", target_bir_lowering=False, debug=False,
                   num_devices=ncores)
    x_d = nc.dram_tensor("x", [N, D], bf16, kind="ExternalInput")
    xT_d = nc.dram_tensor("xT", [D, N], bf16, kind="ExternalInput")
    w_d = nc.dram_tensor("w", [CS, D], bf16, kind="ExternalInput")
    wT_d = nc.dram_tensor("wT", [D, CS], bf16, kind="ExternalInput")
    lab_d = nc.dram_tensor("lab", [P, RT], i32, kind="ExternalInput")
    msk_d = nc.dram_tensor("msk", [P, RT], f32, kind="ExternalInput")
    out_d = nc.dram_tensor("out", [1, 1], f32, kind="ExternalOutput")

    mult = mybir.AluOpType.mult
    add = mybir.AluOpType.add
    sub = mybir.AluOpType.subtract
    amax = mybir.AluOpType.max
    amin = mybir.AluOpType.min
    Exp = mybir.ActivationFunctionType.Exp
    Ln = mybir.ActivationFunctionType.Ln
    DR = mybir.MatmulPerfMode.DoubleRow
    NW = CS // P       # 32 weight chunks of 128 classes
    XG = RT // 4       # 16 x groups (512 rows each)
    WG = 4             # w groups (1024 classes each, 8 chunks)
    QB = RT // 4       # quarter of the row-tile columns (16)

    with tile.TileContext(nc) as tc:
        with tc.tile_pool(name="persist", bufs=1) as persist, \
             tc.tile_pool(name="dram", bufs=1, space="DRAM") as dram, \
             tc.tile_pool(name="io", bufs=3) as io, \
             tc.tile_pool(name="big", bufs=3) as big, \
             tc.tile_pool(name="wgp", bufs=8) as wgp, \
             tc.tile_pool(name="xhp", bufs=8) as xhp, \
             tc.tile_pool(name="small", bufs=4) as small, \
             tc.tile_pool(name="pmm", bufs=2, space="PSUM") as pmm:

            def T(shape, name, dtype=f32):
                return persist.tile(shape, dtype, name=name)

            ones = T([P, 1], "ones")
            nc.vector.memset(ones[:], 1.0)

            labs = T([P, RT], "labs", dtype=i32)
            nc.gpsimd.dma_start(out=labs[:], in_=lab_d[:, :])
            msks = T([P, RT], "msks")
            nc.gpsimd.dma_start(out=msks[:], in_=msk_d[:, :])

            sumexp = T([P, RT], "sumexp")
            cdot = T([P, RT], "cdot")
            ctl = T([P, RT], "ctl")
            marg = T([P, RT], "marg")
            aloc = T([P, RT], "aloc")
            bloc = T([P, RT], "bloc")
            ssw_all = T([P, NW], "ssw_all")
            ssx_all = T([P, RT], "ssx_all")
            ssg_all = T([P, RT], "ssg_all")
            dotg_all = T([P, RT], "dotg_all")
            sxs_all = T([P, RT], "sxs_all")   # 1 / ||x_row||
            wsc_all = T([P, NW], "wsc_all")   # 16 / ||w_class||
            ar_ins = [dram.tile([P, 2 * QB], f32, name=f"ar_in{q}")
                      for q in range(4)]
            ar_outs = [dram.tile([P, 2 * QB], f32, name=f"ar_out{q}")
                       for q in range(4)]


            def rsqrt_newton(dst, src, width, final_scale=None, iters=2):
                # rsqrt on DVE: fast-inverse-sqrt bit seed (scale-free,
                # ~3.4% error) + Newton; 2 iterations reach ~5e-6.  Keeps
                # Ln/Rsqrt tables off the Scalar engine entirely.
                y = small.tile([P, width], f32, name="nwy")
                yi = y[:].bitcast(i32)
                nc.vector.tensor_scalar(
                    out=yi, in0=src.bitcast(i32), scalar1=1, scalar2=None,
                    op0=mybir.AluOpType.logical_shift_right)
                nc.vector.tensor_scalar(
                    out=yi, in0=yi, scalar1=-1, scalar2=0x5F3759DF,
                    op0=mult, op1=add)
                t = small.tile([P, width], f32, name="nwt")
                for it in range(iters):
                    nc.vector.tensor_tensor(out=t[:], in0=y[:], in1=y[:],
                                            op=mult)
                    nc.vector.tensor_tensor(out=t[:], in0=t[:], in1=src,
                                            op=mult)
                    nc.vector.tensor_scalar(out=t[:], in0=t[:],
                                            scalar1=-0.5, scalar2=1.5,
                                            op0=mult, op1=add)
                    if it == iters - 1:
                        if final_scale is not None:
                            nc.vector.scalar_tensor_tensor(
                                out=dst, in0=y[:], scalar=final_scale,
                                in1=t[:], op0=mult, op1=mult)
                        else:
                            nc.vector.tensor_tensor(out=dst, in0=y[:],
                                                    in1=t[:], op=mult)
                    else:
                        nc.vector.tensor_tensor(out=y[:], in0=y[:],
                                                in1=t[:], op=mult)

            # fp8 transposed, normalized, 16x-scaled w (class order permuted;
            # softmax is order-invariant), one tile per 512-class ct block:
            # whT8[ct][p, dc, c] = 16*w_hat[class(ct,c), dc*128 + p].
            whT8 = [T([P, 4, 512], f"whT8_{ct}", dtype=f8)
                    for ct in range(CT)]

            xh8_tiles = {}
            xt_tiles = {}
            xtT_tiles = {}
            xnat_tiles = {}
            blk_starts = {1: 0, 3: 2, 7: 4, 11: 8, 15: 12}

            def emit_xT_load(g2):
                # transposed stationary source: two x groups per DMA.
                # xh8[g][p, dc, m] = x[row(m) = g*512 + m, dc*128 + p] fp8
                xtT2 = io.tile([P, 4, 1024], bf16, name="xtT2", tag="xtT2",
                               bufs=2)
                xtT_tiles[g2] = xtT2
                nc.sync.dma_start(
                    out=xtT2[:],
                    in_=xT_d.rearrange("(dc p) r -> p dc r", p=P)[
                        :, :, g2 * 1024:(g2 + 1) * 1024])

            def emit_xcast(g2):
                xtT2 = xtT_tiles.pop(g2)
                for b in range(2):
                    g = 2 * g2 + b
                    xh8 = xhp.tile([P, 4, 512], f8, name="xh8")
                    xh8_tiles[g] = xh8
                    nc.vector.tensor_copy(
                        out=xh8[:], in_=xtT2[:, :, b * 512:(b + 1) * 512])

            def emit_xnat_load(g2):
                # natural layout for row norms + label dots
                xt2 = io.tile([P, 8, D], bf16, name="xt2", tag="xt2",
                              bufs=3)
                xnat_tiles[g2] = xt2
                nc.sync.dma_start(
                    out=xt2[:].rearrange("p (b a) d -> p b a d", b=2),
                    in_=x_d[g2 * 8 * P:(g2 + 1) * 8 * P, :].rearrange(
                        "(b p a) d -> p b a d", b=2, a=4))

            def emit_xnorm(g2):
                xt2 = xnat_tiles.pop(g2)
                for b in range(2):
                    g = 2 * g2 + b
                    xt_tiles[g] = (xt2, 4 * b)
                    for a in range(4):
                        t = g * 4 + a
                        xjk = big.tile([P, D], bf16, name="junk", bufs=2)
                        nc.vector.scalar_tensor_tensor(
                            out=xjk[:], in0=xt2[:, 4 * b + a], scalar=1.0,
                            in1=xt2[:, 4 * b + a],
                            op0=mult, op1=mult,
                            accum_out=ssx_all[:, t:t + 1])

            def emit_gather(g):
                xt2, boff = xt_tiles.pop(g)
                for a in range(4):
                    t = g * 4 + a
                    wg_t = wgp.tile([P, D], bf16, name="wg")
                    nc.gpsimd.indirect_dma_start(
                        out=wg_t[:], out_offset=None, in_=w_d[:, :],
                        in_offset=bass.IndirectOffsetOnAxis(
                            ap=labs[:, t:t + 1], axis=0))
                    gjk = big.tile([P, D], bf16, name="gjunk", bufs=2)
                    nc.vector.scalar_tensor_tensor(
                        out=gjk[:], in0=wg_t[:], scalar=1.0, in1=wg_t[:],
                        op0=mult, op1=mult,
                        accum_out=ssg_all[:, t:t + 1])
                    gjk2 = big.tile([P, D], bf16, name="gjunk2", bufs=2)
                    nc.vector.scalar_tensor_tensor(
                        out=gjk2[:], in0=wg_t[:], scalar=1.0,
                        in1=xt2[:, boff + a],
                        op0=mult, op1=mult,
                        accum_out=dotg_all[:, t:t + 1])

            def emit_sxs_batch(g):
                g_lo = blk_starts[g]
                blk = slice(g_lo * 4, (g + 1) * 4)
                nblk = (g + 1 - g_lo) * 4
                rsqrt_newton(sxs_all[:, blk], ssx_all[:, blk], nblk)

            def emit_quarter(qt):
                # corrections + allreduce for rows of column quarter qt
                cl = slice(qt * QB, (qt + 1) * QB)
                gsc = small.tile([P, QB], f32, name="gsc")
                rsqrt_newton(gsc[:], ssg_all[:, cl], QB)
                cd = cdot[:, cl]
                nc.vector.tensor_tensor(out=cd, in0=dotg_all[:, cl],
                                        in1=gsc[:], op=mult)
                nc.vector.tensor_tensor(out=cd, in0=cd, in1=sxs_all[:, cl],
                                        op=mult)
                nc.vector.tensor_scalar(out=ctl[:, cl], in0=cd,
                                        scalar1=(-1.0 + EPS),
                                        scalar2=(1.0 - EPS),
                                        op0=amax, op1=amin)
                negc2 = small.tile([P, QB], f32, name="negc2")
                nc.vector.scalar_tensor_tensor(out=negc2[:], in0=ctl[:, cl],
                                               scalar=-1.0, in1=ctl[:, cl],
                                               op0=mult, op1=mult)
                uu = small.tile([P, QB], f32, name="uu")
                nc.vector.tensor_scalar_add(out=uu[:], in0=negc2[:],
                                            scalar1=1.0)
                # sinsq = sin(M) * sqrt(uu) = sin(M) * uu * rsqrt(uu)
                ru = small.tile([P, QB], f32, name="ru")
                rsqrt_newton(ru[:], uu[:], QB)
                sinsq = small.tile([P, QB], f32, name="sinsq")
                nc.vector.scalar_tensor_tensor(out=sinsq[:], in0=uu[:],
                                               scalar=SIN_M, in1=ru[:],
                                               op0=mult, op1=mult)
                nc.vector.scalar_tensor_tensor(out=marg[:, cl],
                                               in0=ctl[:, cl],
                                               scalar=COS_M, in1=sinsq[:],
                                               op0=mult, op1=sub)
                e1 = small.tile([P, QB], f32, name="e1")
                nc.scalar.activation(out=e1[:], in_=marg[:, cl], func=Exp,
                                     scale=S_SCALE)
                e2 = small.tile([P, QB], f32, name="e2")
                nc.scalar.activation(out=e2[:], in_=ctl[:, cl], func=Exp,
                                     scale=S_SCALE)
                d12 = small.tile([P, QB], f32, name="d12")
                nc.vector.scalar_tensor_tensor(out=d12[:], in0=e1[:],
                                               scalar=1.0, in1=e2[:],
                                               op0=mult, op1=sub)
                corr = small.tile([P, QB], f32, name="corr")
                nc.vector.tensor_tensor(out=corr[:], in0=d12[:],
                                        in1=msks[:, cl], op=mult)
                nc.vector.tensor_tensor(out=aloc[:, cl], in0=sumexp[:, cl],
                                        in1=corr[:], op=add)
                nc.vector.scalar_tensor_tensor(out=bloc[:, cl],
                                               in0=marg[:, cl],
                                               scalar=S_SCALE,
                                               in1=msks[:, cl],
                                               op0=mult, op1=mult)
                ar_i, ar_o = ar_ins[qt], ar_outs[qt]
                nc.gpsimd.dma_start(out=ar_i[:, 0:QB], in_=aloc[:, cl])
                nc.gpsimd.dma_start(out=ar_i[:, QB:2 * QB],
                                    in_=bloc[:, cl])
                nc.gpsimd.collective_compute(
                    "AllReduce", add,
                    replica_groups=[list(range(ncores))],
                    ins=[ar_i[:].opt()], outs=[ar_o[:].opt()])

            # hoist the first 4 x groups (transposed halves first — they
            # gate the first matmuls); natural-layout norms are emitted
            # after the W chain so the DVE queue front stays on the
            # first-matmul critical path
            emit_xT_load(0)
            emit_xT_load(1)
            emit_xnat_load(0)
            emit_xnat_load(1)
            emit_xcast(0)
            emit_xcast(1)

            taccs_all = {}

            def emit_block_pass(tiles, ct4):
                for t in tiles:
                    if t not in taccs_all:
                        taccs_all[t] = small.tile([P, 2], f32,
                                                  name="accs", bufs=36)
                for t in tiles:
                    g2, a = t // 4, t % 4
                    ps = pmm.tile([P, 2048], f32, name="ps")
                    for q in range(4):
                        ct = ct4 * 4 + q
                        for b in range(2):
                            nc.tensor.matmul(
                                out=ps[:, q * 512:(q + 1) * 512],
                                lhsT=xh8_tiles[g2][:, 2 * b:2 * b + 2,
                                                   a * 128:(a + 1) * 128],
                                rhs=whT8[ct][:, 2 * b:2 * b + 2, :],
                                start=(b == 0), stop=(b == 1),
                                perf_mode=DR)
                    nc.scalar.activation(
                        out=ps[:], in_=ps[:], func=Exp,
                        scale=sxs_all[:, t:t + 1],
                        accum_out=taccs_all[t][:, ct4:ct4 + 1])

            def emit_block_reduce(tiles):
                for t in tiles:
                    acc = taccs_all.pop(t)
                    nc.vector.tensor_tensor(
                        out=sumexp[:, t:t + 1], in0=acc[:, 0:1],
                        in1=acc[:, 1:2], op=add)

            # ---- stage W: load natural w for class norms and transposed w
            #      for the matmul operand; scale columns to fp8 via a
            #      partition-broadcast multiply (no on-device transposes) ----
            wscrow = T([1, CS], "wscrow")
            for gw in range(WG):
                wt = io.tile([P, 8, D], bf16, name="wt", tag="wt", bufs=2)
                # class c = gw*1024 + p*8 + a  -> 8KB contiguous/partition
                nc.scalar.dma_start(
                    out=wt[:],
                    in_=w_d[gw * 8 * P:(gw + 1) * 8 * P, :].rearrange(
                        "(p a) d -> p a d", a=8))
                wTt = io.tile([P, 4, 1024], bf16, name="wTt", tag="wTt",
                              bufs=2)
                nc.sync.dma_start(
                    out=wTt[:],
                    in_=wT_d.rearrange("(dc p) c -> p dc c", p=P)[
                        :, :, gw * 1024:(gw + 1) * 1024])
                for a in range(8):
                    j = gw * 8 + a
                    wjk = big.tile([P, D], bf16, name="junk", bufs=2)
                    nc.vector.scalar_tensor_tensor(
                        out=wjk[:], in0=wt[:, a], scalar=1.0, in1=wt[:, a],
                        op0=mult, op1=mult,
                        accum_out=ssw_all[:, j:j + 1])
                gsl = slice(gw * 8, (gw + 1) * 8)
                wssc = small.tile([P, 8], f32, name="wssc")
                nc.vector.tensor_scalar_max(out=wssc[:],
                                            in0=ssw_all[:, gsl],
                                            scalar1=SS_FLOOR)
                rsqrt_newton(wsc_all[:, gsl], wssc[:], 8, final_scale=W8)
                # scatter this group's scales into the class-ordered row,
                # then physically replicate it across partitions
                nc.gpsimd.dma_start(
                    out=wscrow[0:1, gw * 1024:(gw + 1) * 1024].rearrange(
                        "one (p a) -> one p a", p=P),
                    in_=wsc_all[:, gsl])
                wscb = big.tile([P, 1024], f32, name="wscb", bufs=2)
                nc.gpsimd.partition_broadcast(
                    wscb[:], wscrow[0:1, gw * 1024:(gw + 1) * 1024])
                for h in range(2):
                    ct = gw * 2 + h
                    for dc in range(4):
                        nc.vector.tensor_tensor(
                            out=whT8[ct][:, dc, :],
                            in0=wTt[:, dc, h * 512:(h + 1) * 512],
                            in1=wscb[:, h * 512:(h + 1) * 512],
                            op=mult)
                if gw == 1:
                    emit_xnorm(0)
                    emit_sxs_batch(1)
                    emit_xnorm(1)
                    emit_sxs_batch(3)
                    emit_block_pass(list(range(8)), 0)
            emit_block_pass(list(range(8)), 1)
            emit_block_reduce(list(range(8)))



            # ---- main loop: x loads 4 groups ahead, gathers 1 behind,
            #      sxs batches 2 iterations ahead of their block ----
            batch_at = {5: 7, 9: 11, 13: 15}
            blk2 = {3: 2, 7: 4, 11: 8, 15: 12}
            qtr_at = {5: 0, 9: 1, 13: 2}
            for g in range(XG):
                if g % 2 == 0 and g + 4 < XG:
                    g2n = (g + 4) // 2
                    emit_xT_load(g2n)
                    emit_xnat_load(g2n)
                    emit_xcast(g2n)
                    emit_xnorm(g2n)
                if g in batch_at:
                    emit_sxs_batch(batch_at[g])
                if g in blk2:
                    tiles = list(range(blk2[g] * 4, (g + 1) * 4))
                    for ct4 in range(2):
                        emit_block_pass(tiles, ct4)
                    emit_block_reduce(tiles)
                if g >= 1:
                    emit_gather(g - 1)
                if g in qtr_at:
                    emit_quarter(qtr_at[g])
            emit_gather(XG - 1)
            emit_quarter(3)

            # ---- combine quarters and reduce to the scalar mean.  The
            # q0-q2 math is emitted first so it runs (after the exp stream,
            # one Ln table load) while quarter 3's AllReduce is in flight;
            # only q3's [P,16] math trails the final AR. ----
            gg = T([P, 2 * RT], "gg")
            nllq = T([P, RT], "nllq")
            rsums = T([P, 4], "rsums")
            for qt in range(4):
                ql = slice(qt * QB, (qt + 1) * QB)
                nc.gpsimd.dma_start(out=gg[:, qt * QB:(qt + 1) * QB],
                                    in_=ar_outs[qt][:, 0:QB])
                nc.gpsimd.dma_start(
                    out=gg[:, RT + qt * QB:RT + (qt + 1) * QB],
                    in_=ar_outs[qt][:, QB:2 * QB])
                atq = small.tile([P, QB], f32, name="atq")
                nc.vector.tensor_scalar_add(out=atq[:], in0=gg[:, ql],
                                            scalar1=-NPAD)
                lnaq = small.tile([P, QB], f32, name="lnaq")
                nc.scalar.activation(out=lnaq[:], in_=atq[:], func=Ln)
                nc.vector.scalar_tensor_tensor(
                    out=nllq[:, ql], in0=lnaq[:], scalar=1.0,
                    in1=gg[:, RT + qt * QB:RT + (qt + 1) * QB],
                    op0=mult, op1=sub)
                nc.vector.reduce_sum(out=rsums[:, qt:qt + 1],
                                     in_=nllq[:, ql],
                                     axis=mybir.AxisListType.X)
            rsum = T([P, 1], "rsum")
            nc.vector.reduce_sum(out=rsum[:], in_=rsums[:],
                                 axis=mybir.AxisListType.X)
            pf = pmm.tile([P, 2048], f32, name="ps")
            nc.tensor.matmul(out=pf[:1, :1], lhsT=rsum[:, :1],
                             rhs=ones[:, :1], start=True, stop=True)
            res = T([1, 1], "res")
            nc.vector.tensor_scalar_mul(out=res[:], in0=pf[:1, :1],
                                        scalar1=1.0 / float(N))
            nc.gpsimd.dma_start(out=out_d[:, :], in_=res[:])

    nc.compile()
    return nc


def _get_nc():
    if "nc" not in _CACHE:
        _CACHE["nc"] = _build()
    return _CACHE["nc"]


def kernel(prev_output, weight, labels, **trace_kwargs):
    bf = ml_dtypes.bfloat16
    x = np.ascontiguousarray(prev_output, dtype=np.float32)
    xb16 = x.astype(bf)
    xT16 = x.T.astype(bf)
    w = np.ascontiguousarray(weight, dtype=np.float32)
    lab = np.asarray(labels).astype(np.int64)

    wpad = np.zeros((CPAD, D), dtype=bf)
    wpad[:C] = w.astype(bf)

    in_maps = []
    for k in range(NCORES):
        lo = k * CS
        loc = (lab - lo).astype(np.int64)
        own = (loc >= 0) & (loc < CS)
        locc = np.clip(loc, 0, CS - 1).astype(np.int32)
        # row r = g*512 + p*4 + a maps to [p, t=g*4+a]
        lab2 = locc.reshape(RT // 4, P, 4).transpose(1, 0, 2).reshape(P, RT)
        msk2 = own.astype(np.float32).reshape(RT // 4, P, 4) \
                  .transpose(1, 0, 2).reshape(P, RT)
        in_maps.append({
            "x": xb16,
            "xT": xT16,
            "w": wpad[lo:lo + CS],
            "wT": np.ascontiguousarray(wpad[lo:lo + CS].T),
            "lab": np.ascontiguousarray(lab2),
            "msk": np.ascontiguousarray(msk2),
        })

    nc = _get_nc()
    res = run_bass_kernel_spmd(nc, in_maps, core_ids=list(range(NCORES)),
                               **trace_kwargs)
    if trace_kwargs:
        _CACHE["last_results"] = res
    return np.float32(res.results[0]["out"].reshape(())[()])


if __name__ == "__main__":
    rng = np.random.default_rng(0)
    x = rng.standard_normal((N, D), dtype=np.float32)
    w = rng.standard_normal((C, D), dtype=np.float32) * 0.01
    lab = rng.integers(0, C, N)
    got = kernel(x, w, lab)
    xh = x / np.maximum(np.linalg.norm(x, axis=1, keepdims=True), 1e-12)
    wh = w / np.maximum(np.linalg.norm(w, axis=1, keepdims=True), 1e-12)
    cos = (xh @ wh.T).astype(np.float64)
    th = np.arccos(np.clip(cos[np.arange(N), lab], -1 + EPS, 1 - EPS))
    ml = np.cos(th + M_MARGIN)
    logits = cos * S_SCALE
    tgt = ml * S_SCALE
    lse = np.log(np.exp(logits).sum(1) - np.exp(logits[np.arange(N), lab])
                 + np.exp(tgt))
    want = (lse - tgt).mean()
    print("got", got, "want", want, "relerr", abs(got - want) / abs(want))
